# revision 37
# baseline (speedup 1.0000x reference)
"""Trainium2 Bass kernel: multi-head attention with Toeplitz relative bias.

Problem: B=16, L=1024, F=512, H=8, D=64 ViT patch attention.
Sharding: data-parallel over batch, 2 batches per core across 8 cores.

Device-side design (per core, fully unrolled Tile program):
  - Host pre-packs every DRAM operand partition-major so each DMA descriptor
    covers a full 4-8KB partition row (small-descriptor overhead dominated
    the staging cost otherwise).
  - qT/kT computed transposed ([fout, L], W stationary); head pair stacked
    on partitions (64 rows each); scores use K=64 matmuls with matching
    partition bases, so kT needs no zero padding or memset.
  - v computed natural ([L, fout], xT stationary, bv via ones-row matmul).
  - Scores computed transposed [k, q] (k on partitions) so attn@v needs no
    transpose of the attention matrix. ACT does exp; DVE multiplies in the
    host-precomputed exp(bias) (fp16, 2x DVE mode).
  - attn@v in natural [q, d] layout with exp chunks as the stationary
    operand; softmax denominators accumulate into column 64 via a
    ones-column in vA, so normalization is a per-partition divide fused into
    one DVE tensor_scalar op.
  - The head loop is software-pipelined at (head, batch)-window granularity:
    window w computes scores/exp for pair (h,b) = (w//2, w%2) while the
    PREVIOUS pair's attn@v drains on the PE, so ACT (the ~1.1us/tile exp
    engine) and the PE both stay continuously busy.  QK projections are fed
    through a background queue one 4-matmul chain per slot.
  - x_attn is PE-transposed (identity trick) for the output projection; bo
    folded in via a ones-row matmul.
  - No max-subtraction in softmax: |scores| <~ 1.5 by construction.
"""

import sys

for _p in ("/opt/trn_rl_repo",):
    if _p not in sys.path:
        sys.path.insert(0, _p)

import numpy as np
import ml_dtypes

import concourse.bass as bass
import concourse.mybir as mybir
import concourse.tile as tile
from concourse import bacc
from concourse.bass_utils import run_bass_kernel_spmd
from concourse.masks import make_identity

B, L, F, H, D = 16, 1024, 512, 8, 64
NX, NY = 32, 32
NCORES = 8
BPC = B // NCORES  # batches per core
FP32 = mybir.dt.float32
BF16 = mybir.dt.bfloat16
FP16 = mybir.dt.float16
Exp = mybir.ActivationFunctionType.Exp
Identity = mybir.ActivationFunctionType.Identity
Mult = mybir.AluOpType.mult


def _build():
    nc = bacc.Bacc("TRN2", target_bir_lowering=False, debug=False)

    xqT_d = nc.dram_tensor("xqT", [BPC, 128, 4 * L], BF16, kind="ExternalInput").ap()
    xkvT_d = nc.dram_tensor("xkvT", [BPC, 128, 4 * L], BF16, kind="ExternalInput").ap()
    Wq_d = nc.dram_tensor("Wq", [128, 4 * F], BF16, kind="ExternalInput").ap()
    Wk_d = nc.dram_tensor("Wk", [128, 4 * F], BF16, kind="ExternalInput").ap()
    Wv_d = nc.dram_tensor("Wv", [128, 4 * F], BF16, kind="ExternalInput").ap()
    Wo_d = nc.dram_tensor("Wo", [128, 4 * F], BF16, kind="ExternalInput").ap()
    bq_d = nc.dram_tensor("bq", [F], FP32, kind="ExternalInput").ap()
    bk_d = nc.dram_tensor("bk", [F], FP32, kind="ExternalInput").ap()
    bv_d = nc.dram_tensor("bv", [128, F], BF16, kind="ExternalInput").ap()
    bo_d = nc.dram_tensor("bo", [128, F], BF16, kind="ExternalInput").ap()
    biasT_d = nc.dram_tensor("biasT", [H, 2, 128, 4 * L], FP16, kind="ExternalInput").ap()
    ones_d = nc.dram_tensor("ones", [128, 128], BF16, kind="ExternalInput").ap()
    out_d = nc.dram_tensor("out", [BPC, L, F], BF16, kind="ExternalOutput").ap()

    with tile.TileContext(nc) as tc:
        with (
            tc.tile_pool(name="const", bufs=1) as cpool,
            tc.tile_pool(name="xin", bufs=2) as xpool,
            tc.tile_pool(name="qkv", bufs=2) as qpool,
            tc.tile_pool(name="bias", bufs=3) as bpool,
            tc.tile_pool(name="work", bufs=2) as wpool,
            tc.tile_pool(name="exp", bufs=18) as epool,
            tc.tile_pool(name="es", bufs=3) as espool,
            tc.tile_pool(name="psA", bufs=3, space="PSUM") as psA,
            tc.tile_pool(name="psU", bufs=2, space="PSUM") as psU,
        ):
            # ---- constants ----
            Wv_s = cpool.tile([128, 4 * F], BF16, tag="Wv")
            Wq_s = cpool.tile([128, 4 * F], BF16, tag="Wq")
            Wk_s = cpool.tile([128, 4 * F], BF16, tag="Wk")
            Wo_s = cpool.tile([128, 4 * F], BF16, tag="Wo")

            bias_tiles = {}

            def stage_bias(h):
                tiles = []
                for hh in range(2):
                    bt = bpool.tile([128, 4 * L], FP16, tag="bias")
                    nc.sync.dma_start(out=bt[:], in_=biasT_d[h, hh])
                    tiles.append(bt)
                bias_tiles[h] = tiles

            bq_s = cpool.tile([128, 4], FP32, tag="bq")
            bk_s = cpool.tile([128, 4], FP32, tag="bk")

            qT, kT, vA, xan, xatT, xq, xkv = [], [], [], [], [], [], []
            for b in range(BPC):
                xq_t = xpool.tile([128, 4 * L], BF16, tag="xq")
                xkv_t = xpool.tile([128, 4 * L], BF16, tag="xkv")
                nc.sync.dma_start(out=xkv_t[:], in_=xkvT_d[b])
                if b == 0:
                    nc.sync.dma_start(out=Wq_s[:], in_=Wq_d)
                nc.sync.dma_start(out=xq_t[:], in_=xqT_d[b])
                if b == 0:
                    nc.sync.dma_start(out=Wk_s[:], in_=Wk_d)
                    for b_s, b_d in ((bq_s, bq_d), (bk_s, bk_d)):
                        nc.sync.dma_start(
                            out=b_s[:], in_=b_d.rearrange("(c p) -> p c", p=128)
                        )
                xq.append(xq_t)
                xkv.append(xkv_t)
                if b == 0:
                    stage_bias(0)
                    nc.sync.dma_start(out=Wv_s[:], in_=Wv_d)
                    ones_s = cpool.tile([128, 128], BF16, tag="ones")
                    nc.sync.dma_start(out=ones_s[:], in_=ones_d)
                    bv_s = cpool.tile([128, F], BF16, tag="bv")
                    nc.sync.dma_start(out=bv_s[:], in_=bv_d)
                qT_t = qpool.tile([128, 4 * L], BF16, tag="qT")
                kT_t = qpool.tile([128, 4 * L], BF16, tag="kT")
                vA_t = qpool.tile([128, 8 * 8 * 65], FP16, tag="vA")
                xan_t = qpool.tile([128, 8 * F], BF16, tag="xan")
                xatT_t = qpool.tile([128, 4 * L], BF16, tag="xatT")
                qT.append(qT_t)
                kT.append(kT_t)
                vA.append(vA_t)
                xan.append(xan_t)
                xatT.append(xatT_t)
            stage_bias(1)
            load_late = [(Wo_s, Wo_d)]
            bo_s = cpool.tile([128, F], BF16, tag="bo")
            ident = cpool.tile([128, 128], BF16, tag="ident")
            make_identity(nc, ident[:])
            nc.sync.dma_start(out=Wo_s[:], in_=Wo_d)
            nc.sync.dma_start(out=bo_s[:], in_=bo_d)

            # ---- background PE work generators ----
            def v_proj_steps(b):
                # ones column for the softmax denominator accumulation
                nc.gpsimd.memset(
                    vA[b][:].rearrange("p (t h w) -> p t h w", t=8, h=8)[:, :, :, 64:65],
                    1.0,
                )
                # v natural (+bv via ones-row matmul): xT stationary, Wv moving
                for lt in range(8):
                    pv = psA.tile([128, 512], FP32, tag="ps")
                    for kc in range(4):
                        nc.tensor.matmul(
                            pv[:],
                            xkv[b][:, kc * L + lt * 128 : kc * L + (lt + 1) * 128],
                            Wv_s[:, kc * F : (kc + 1) * F],
                            start=(kc == 0),
                            stop=False,
                        )
                    nc.tensor.matmul(pv[:], ones_s[:], bv_s[:], start=False, stop=True)
                    nc.scalar.activation(
                        vA[b][:, lt * 520 : (lt + 1) * 520].rearrange(
                            "p (h w) -> p h w", h=8
                        )[:, :, 0:64],
                        pv[:].rearrange("p (h w) -> p h w", h=8),
                        Identity,
                        bias=0.0,
                    )
                    yield

            def qk_proj_steps(fo, b):
                # qT/kT transposed [fout, L]; head pair stacked on partitions
                for which, w_s, b_s, x_t, dst in (
                    ("q", Wq_s, bq_s, xq[b], qT[b]),
                    ("k", Wk_s, bk_s, xkv[b], kT[b]),
                ):
                    pq = psA.tile([128, 2 * 512], FP32, tag="ps")
                    for lc in range(2):
                        for kc in range(4):
                            nc.tensor.matmul(
                                pq[:, lc * 512 : (lc + 1) * 512],
                                w_s[:, kc * F + fo * 128 : kc * F + (fo + 1) * 128],
                                x_t[:, kc * L + lc * 512 : kc * L + (lc + 1) * 512],
                                start=(kc == 0),
                                stop=(kc == 3),
                            )
                        if lc == 0:
                            yield
                    nc.vector.tensor_scalar_add(
                        dst[:, fo * L : (fo + 1) * L], pq[:], b_s[:, fo : fo + 1]
                    )
                    yield

            def transpose_steps(b, c):
                # transpose x_attn chunk c (heads 2c, 2c+1) via identity trick
                for qt in range(8):
                    pt = psA.tile([128, 512], BF16, tag="ps")
                    nc.tensor.transpose(
                        pt[:, 0:128],
                        xan[b][:, qt * F + c * 128 : qt * F + (c + 1) * 128],
                        ident[:],
                    )
                    nc.vector.tensor_copy(
                        xatT[b][:, c * L + qt * 128 : c * L + (qt + 1) * 128],
                        pt[:, 0:128],
                    )
                    if qt % 4 == 3:
                        yield

            def outproj_steps(b):
                for lt in range(8):
                    po = psA.tile([128, 512], FP32, tag="ps")
                    for c in range(4):
                        nc.tensor.matmul(
                            po[:],
                            xatT[b][:, c * L + lt * 128 : c * L + (lt + 1) * 128],
                            Wo_s[:, c * F : (c + 1) * F],
                            start=(c == 0),
                            stop=False,
                        )
                    nc.tensor.matmul(po[:], ones_s[:], bo_s[:], start=False, stop=True)
                    os_t = wpool.tile([128, 512], BF16, tag="os")
                    nc.scalar.copy(os_t[:], po[:])
                    nc.sync.dma_start(
                        out=out_d[b, lt * 128 : (lt + 1) * 128, :], in_=os_t[:]
                    )
                    yield

            bg = []

            def bg_step(n=1):
                for _ in range(n):
                    while bg:
                        try:
                            next(bg[0])
                            break
                        except StopIteration:
                            bg.pop(0)
                    else:
                        return

            # ---- window-pipelined phase B ----
            # window w: scores/exp/bias-mult for (h, b) = (w//2, w%2);
            # attn@v (+ per-qt softmax normalize) for the previous pair.
            for _ in qk_proj_steps(0, 0):
                pass
            bg.append(qk_proj_steps(0, 1))
            bg.append(v_proj_steps(0))
            bg.append(v_proj_steps(1))
            pending_proj = {1: (1, 0), 2: (1, 1), 5: (2, 0), 6: (2, 1),
                            9: (3, 0), 10: (3, 1)}

            def attnv_qt(h, b, exs, qt):
                U = psU.tile([128, 65], FP32, tag="u")
                for kt in range(8):
                    nc.tensor.matmul(
                        U[:],
                        exs[kt][:, qt * 128 : (qt + 1) * 128],
                        vA[b][:, kt * 520 + h * 65 : kt * 520 + (h + 1) * 65],
                        start=(kt == 0),
                        stop=(kt == 7),
                    )
                rc = wpool.tile([128, 1], FP32, tag="rc")
                nc.vector.reciprocal(rc[:], U[:, 64:65])
                nc.vector.tensor_scalar(
                    xan[b][:, qt * F + h * 64 : qt * F + (h + 1) * 64],
                    U[:, 0:64],
                    rc[:],
                    None,
                    op0=Mult,
                )

            prev = None  # (h, b, [ex tiles]) of the pair being drained
            for w in range(2 * H):
                h, b = w // 2, w % 2
                hp = (h % 2) * 64
                hc = (h // 2) * L
                if w in pending_proj:
                    bg.append(qk_proj_steps(*pending_proj[w]))
                if b == 0 and h >= 1 and h + 1 < H:
                    stage_bias(h + 1)
                exs = []
                for kt in range(8):
                    ps = psA.tile([128, 2 * 512], FP32, tag="ps")
                    for qc in range(2):
                        nc.tensor.matmul(
                            ps[:, qc * 512 : (qc + 1) * 512],
                            kT[b][hp : hp + 64, hc + kt * 128 : hc + (kt + 1) * 128],
                            qT[b][hp : hp + 64, hc + qc * 512 : hc + (qc + 1) * 512],
                            start=True,
                            stop=True,
                        )
                    es = espool.tile([128, 2 * 512], FP16, tag="es")
                    nc.scalar.activation(es[:], ps[:], Exp)
                    ex = epool.tile([128, 2 * 512], FP16, tag="ex")
                    # 2 of 8 bias multiplies ride the otherwise-idle gpsimd
                    eng = nc.gpsimd if kt in (2, 5) else nc.vector
                    eng.tensor_tensor(
                        ex[:],
                        es[:],
                        bias_tiles[h][kt // 4][:, (kt % 4) * L : (kt % 4 + 1) * L],
                        Mult,
                    )
                    exs.append(ex)
                    if prev is not None and kt >= 1:
                        attnv_qt(prev[0], prev[1], prev[2], kt - 1)
                    bg_step(2 if w <= 1 else 1)
                if prev is not None:
                    attnv_qt(prev[0], prev[1], prev[2], 7)
                    if prev[0] % 2 == 1:
                        bg.append(transpose_steps(prev[1], (prev[0] - 1) // 2))
                    if prev[0] == H - 1:
                        bg.append(outproj_steps(prev[1]))
                prev = (h, b, exs)

            # drain: attn@v for the last pair (h=H-1, b=1)
            for qt in range(8):
                attnv_qt(prev[0], prev[1], prev[2], qt)
                bg_step(2)
            bg.append(transpose_steps(prev[1], 3))
            bg.append(outproj_steps(prev[1]))
            while bg:
                bg_step()

    nc.compile()
    return nc


_NC = None


def _get_nc():
    global _NC
    if _NC is None:
        _NC = _build()
    return _NC


def _prep_in_maps(inputs):
    bf16 = ml_dtypes.bfloat16
    xq = np.asarray(inputs["inputs_q"], dtype=np.float32)
    xkv = np.asarray(inputs["inputs_kv"], dtype=np.float32)
    Wq = np.asarray(inputs["Wq"], dtype=np.float32) * 0.125
    bq = np.asarray(inputs["bq"], dtype=np.float32) * 0.125
    Wk = np.asarray(inputs["Wk"], dtype=np.float32)
    bk = np.asarray(inputs["bk"], dtype=np.float32)
    Wv = np.asarray(inputs["Wv"], dtype=np.float32)
    Wo = np.asarray(inputs["Wo"], dtype=np.float32)
    bv_pad = np.zeros((128, F), dtype=np.float32)
    bv_pad[0] = np.asarray(inputs["bv"], dtype=np.float32)
    bo_pad = np.zeros((128, F), dtype=np.float32)
    bo_pad[0] = np.asarray(inputs["bo"], dtype=np.float32)
    onesrow = np.zeros((128, 128), dtype=np.float32)
    onesrow[0] = 1.0
    toe = np.asarray(inputs["toeplitz"], dtype=np.float32)

    def pack_x(x):
        # [B, L, F] -> xT [B, F, L] -> partition-major [B, 128, 4*L]
        xT = x.transpose(0, 2, 1).reshape(B, 4, 128, L)
        return np.ascontiguousarray(xT.transpose(0, 2, 1, 3)).reshape(
            B, 128, 4 * L
        ).astype(bf16)

    def pack_w(w):
        # [F, F] -> partition-major [128, 4*F]
        return np.ascontiguousarray(
            w.reshape(4, 128, F).transpose(1, 0, 2)
        ).reshape(128, 4 * F).astype(bf16)

    xqT = pack_x(xq)
    xkvT = pack_x(xkv)
    WqP, WkP, WvP, WoP = pack_w(Wq), pack_w(Wk), pack_w(Wv), pack_w(Wo)

    coords = np.arange(L)
    xi, yi = coords // NY, coords % NY
    dx = xi[:, None] - xi[None, :] + NX
    dy = yi[:, None] - yi[None, :] + NY
    idx = dx * (2 * NY) + dy  # [L(q), L(k)]
    bias = toe[:, idx]  # [H, L(q), L(k)]
    biasT = np.exp(bias.transpose(0, 2, 1))  # [H, L(k), L(q)]
    # partition-major [H, 2, 128, 4*L]
    biasT = np.ascontiguousarray(
        biasT.reshape(H, 2, 4, 128, L).transpose(0, 1, 3, 2, 4)
    ).reshape(H, 2, 128, 4 * L).astype(np.float16)

    in_maps = []
    for i in range(NCORES):
        sl = slice(i * BPC, (i + 1) * BPC)
        in_maps.append(
            {
                "xqT": np.ascontiguousarray(xqT[sl]),
                "xkvT": np.ascontiguousarray(xkvT[sl]),
                "Wq": WqP, "Wk": WkP, "Wv": WvP, "Wo": WoP,
                "bq": bq, "bk": bk,
                "bv": bv_pad.astype(bf16), "bo": bo_pad.astype(bf16),
                "biasT": biasT,
                "ones": onesrow.astype(bf16),
            }
        )
    return in_maps


def _run(inputs, trace=False):
    from concourse.bass_interp import get_hw_module

    nc = _get_nc()
    in_maps = _prep_in_maps(inputs)
    old_m = nc.m
    nc.m = get_hw_module(nc.m)
    try:
        res = run_bass_kernel_spmd(
            nc, in_maps, core_ids=list(range(NCORES)), trace=trace
        )
    finally:
        nc.m = old_m
    out = np.concatenate(
        [np.asarray(r["out"], dtype=np.float32) for r in res.results], axis=0
    )  # [B, L, F]
    return out.reshape(B, L, H, D), res


def kernel(**inputs) -> np.ndarray:
    out, _ = _run(inputs, trace=False)
    return out


# revision 38
# speedup vs baseline: 1.0335x; 1.0335x over previous
"""Trainium2 Bass kernel: multi-head attention with Toeplitz relative bias.

Problem: B=16, L=1024, F=512, H=8, D=64 ViT patch attention.
Sharding: data-parallel over batch, 2 batches per core across 8 cores.

Device-side design (per core, fully unrolled Tile program):
  - Host pre-packs every DRAM operand partition-major so each DMA descriptor
    covers a full 4-8KB partition row (small-descriptor overhead dominated
    the staging cost otherwise).
  - qT/kT computed transposed ([fout, L], W stationary); head pair stacked
    on partitions (64 rows each); scores use K=64 matmuls with matching
    partition bases, so kT needs no zero padding or memset.
  - v computed natural ([L, fout], xT stationary, bv via ones-row matmul).
  - Scores computed transposed [k, q] (k on partitions) so attn@v needs no
    transpose of the attention matrix. ACT does exp; DVE multiplies in the
    host-precomputed exp(bias) (fp16, 2x DVE mode).
  - attn@v in natural [q, d] layout with exp chunks as the stationary
    operand; softmax denominators accumulate into column 64 via a
    ones-column in vA, so normalization is a per-partition divide fused into
    one DVE tensor_scalar op.
  - The head loop is software-pipelined at (head, batch)-window granularity:
    window w computes scores/exp for pair (h,b) = (w//2, w%2) while the
    PREVIOUS pair's attn@v drains on the PE, so ACT (the ~1.1us/tile exp
    engine) and the PE both stay continuously busy.  QK projections are fed
    through a background queue one 4-matmul chain per slot.
  - x_attn is PE-transposed (identity trick) for the output projection; bo
    folded in via a ones-row matmul.
  - No max-subtraction in softmax: |scores| <~ 1.5 by construction.
"""

import sys

for _p in ("/opt/trn_rl_repo",):
    if _p not in sys.path:
        sys.path.insert(0, _p)

import numpy as np
import ml_dtypes

import concourse.bass as bass
import concourse.mybir as mybir
import concourse.tile as tile
from concourse import bacc
from concourse.bass_utils import run_bass_kernel_spmd
from concourse.masks import make_identity

B, L, F, H, D = 16, 1024, 512, 8, 64
NX, NY = 32, 32
NCORES = 8
BPC = B // NCORES  # batches per core
FP32 = mybir.dt.float32
BF16 = mybir.dt.bfloat16
FP16 = mybir.dt.float16
Exp = mybir.ActivationFunctionType.Exp
Identity = mybir.ActivationFunctionType.Identity
Mult = mybir.AluOpType.mult


def _build():
    nc = bacc.Bacc("TRN2", target_bir_lowering=False, debug=False)

    xqT_d = nc.dram_tensor("xqT", [BPC, 128, 4 * L], BF16, kind="ExternalInput").ap()
    xkvT_d = nc.dram_tensor("xkvT", [BPC, 128, 4 * L], BF16, kind="ExternalInput").ap()
    Wq_d = nc.dram_tensor("Wq", [128, 4 * F], BF16, kind="ExternalInput").ap()
    Wk_d = nc.dram_tensor("Wk", [128, 4 * F], BF16, kind="ExternalInput").ap()
    Wv_d = nc.dram_tensor("Wv", [128, 4 * F], BF16, kind="ExternalInput").ap()
    Wo_d = nc.dram_tensor("Wo", [128, 4 * F], BF16, kind="ExternalInput").ap()
    bq_d = nc.dram_tensor("bq", [F], FP32, kind="ExternalInput").ap()
    bk_d = nc.dram_tensor("bk", [F], FP32, kind="ExternalInput").ap()
    bv_d = nc.dram_tensor("bv", [128, F], BF16, kind="ExternalInput").ap()
    bo_d = nc.dram_tensor("bo", [128, F], BF16, kind="ExternalInput").ap()
    biasT_d = nc.dram_tensor("biasT", [H, 2, 128, 4 * L], FP16, kind="ExternalInput").ap()
    ones_d = nc.dram_tensor("ones", [128, 128], BF16, kind="ExternalInput").ap()
    out_d = nc.dram_tensor("out", [BPC, L, F], BF16, kind="ExternalOutput").ap()

    with tile.TileContext(nc) as tc:
        with (
            tc.tile_pool(name="const", bufs=1) as cpool,
            tc.tile_pool(name="xin", bufs=2) as xpool,
            tc.tile_pool(name="qkv", bufs=2) as qpool,
            tc.tile_pool(name="bias", bufs=3) as bpool,
            tc.tile_pool(name="work", bufs=2) as wpool,
            tc.tile_pool(name="exp", bufs=18) as epool,
            tc.tile_pool(name="es", bufs=3) as espool,
            tc.tile_pool(name="psA", bufs=3, space="PSUM") as psA,
            tc.tile_pool(name="psU", bufs=2, space="PSUM") as psU,
        ):
            # ---- constants ----
            Wv_s = cpool.tile([128, 4 * F], BF16, tag="Wv")
            Wq_s = cpool.tile([128, 4 * F], BF16, tag="Wq")
            Wk_s = cpool.tile([128, 4 * F], BF16, tag="Wk")
            Wo_s = cpool.tile([128, 4 * F], BF16, tag="Wo")

            bias_tiles = {}

            def stage_bias(h):
                tiles = []
                for hh in range(2):
                    bt = bpool.tile([128, 4 * L], FP16, tag="bias")
                    nc.sync.dma_start(out=bt[:], in_=biasT_d[h, hh])
                    tiles.append(bt)
                bias_tiles[h] = tiles

            bq_s = cpool.tile([128, 4], FP32, tag="bq")
            bk_s = cpool.tile([128, 4], FP32, tag="bk")

            qT, kT, vA, xan, xatT, xq, xkv = [], [], [], [], [], [], []
            for b in range(BPC):
                xq_t = xpool.tile([128, 4 * L], BF16, tag="xq")
                xkv_t = xpool.tile([128, 4 * L], BF16, tag="xkv")
                nc.sync.dma_start(out=xkv_t[:], in_=xkvT_d[b])
                if b == 0:
                    nc.sync.dma_start(out=Wq_s[:], in_=Wq_d)
                nc.sync.dma_start(out=xq_t[:], in_=xqT_d[b])
                if b == 0:
                    nc.sync.dma_start(out=Wk_s[:], in_=Wk_d)
                    for b_s, b_d in ((bq_s, bq_d), (bk_s, bk_d)):
                        nc.sync.dma_start(
                            out=b_s[:], in_=b_d.rearrange("(c p) -> p c", p=128)
                        )
                xq.append(xq_t)
                xkv.append(xkv_t)
                if b == 0:
                    stage_bias(0)
                    nc.sync.dma_start(out=Wv_s[:], in_=Wv_d)
                    ones_s = cpool.tile([128, 128], BF16, tag="ones")
                    nc.sync.dma_start(out=ones_s[:], in_=ones_d)
                    bv_s = cpool.tile([128, F], BF16, tag="bv")
                    nc.sync.dma_start(out=bv_s[:], in_=bv_d)
                qT_t = qpool.tile([128, 4 * L], BF16, tag="qT")
                kT_t = qpool.tile([128, 4 * L], BF16, tag="kT")
                vA_t = qpool.tile([128, 8 * 8 * 65], FP16, tag="vA")
                xan_t = qpool.tile([128, 8 * F], BF16, tag="xan")
                xatT_t = qpool.tile([128, 4 * L], BF16, tag="xatT")
                qT.append(qT_t)
                kT.append(kT_t)
                vA.append(vA_t)
                xan.append(xan_t)
                xatT.append(xatT_t)
            stage_bias(1)
            load_late = [(Wo_s, Wo_d)]
            bo_s = cpool.tile([128, F], BF16, tag="bo")
            ident = cpool.tile([128, 128], BF16, tag="ident")
            make_identity(nc, ident[:])
            nc.sync.dma_start(out=Wo_s[:], in_=Wo_d)
            nc.sync.dma_start(out=bo_s[:], in_=bo_d)

            # ---- background PE work generators ----
            def v_proj_steps(b):
                # ones column for the softmax denominator accumulation
                nc.gpsimd.memset(
                    vA[b][:].rearrange("p (t h w) -> p t h w", t=8, h=8)[:, :, :, 64:65],
                    1.0,
                )
                # v natural (+bv via ones-row matmul): xT stationary, Wv moving
                for lt in range(8):
                    pv = psA.tile([128, 512], FP32, tag="ps")
                    for kc in range(4):
                        nc.tensor.matmul(
                            pv[:],
                            xkv[b][:, kc * L + lt * 128 : kc * L + (lt + 1) * 128],
                            Wv_s[:, kc * F : (kc + 1) * F],
                            start=(kc == 0),
                            stop=False,
                        )
                    nc.tensor.matmul(pv[:], ones_s[:], bv_s[:], start=False, stop=True)
                    nc.scalar.activation(
                        vA[b][:, lt * 520 : (lt + 1) * 520].rearrange(
                            "p (h w) -> p h w", h=8
                        )[:, :, 0:64],
                        pv[:].rearrange("p (h w) -> p h w", h=8),
                        Identity,
                        bias=0.0,
                    )
                    yield

            def qk_proj_steps(fo, b):
                # qT/kT transposed [fout, L]; head pair stacked on partitions
                for which, w_s, b_s, x_t, dst in (
                    ("q", Wq_s, bq_s, xq[b], qT[b]),
                    ("k", Wk_s, bk_s, xkv[b], kT[b]),
                ):
                    pq = psA.tile([128, 2 * 512], FP32, tag="ps")
                    for lc in range(2):
                        for kc in range(4):
                            nc.tensor.matmul(
                                pq[:, lc * 512 : (lc + 1) * 512],
                                w_s[:, kc * F + fo * 128 : kc * F + (fo + 1) * 128],
                                x_t[:, kc * L + lc * 512 : kc * L + (lc + 1) * 512],
                                start=(kc == 0),
                                stop=(kc == 3),
                            )
                        if lc == 0:
                            yield
                    nc.vector.tensor_scalar_add(
                        dst[:, fo * L : (fo + 1) * L], pq[:], b_s[:, fo : fo + 1]
                    )
                    yield

            def transpose_steps(b, c):
                # transpose x_attn chunk c (heads 2c, 2c+1) via identity trick
                for qt in range(8):
                    pt = psA.tile([128, 512], BF16, tag="ps")
                    nc.tensor.transpose(
                        pt[:, 0:128],
                        xan[b][:, qt * F + c * 128 : qt * F + (c + 1) * 128],
                        ident[:],
                    )
                    nc.vector.tensor_copy(
                        xatT[b][:, c * L + qt * 128 : c * L + (qt + 1) * 128],
                        pt[:, 0:128],
                    )
                    if qt % 4 == 3:
                        yield

            def outproj_steps(b):
                for lt in range(8):
                    po = psA.tile([128, 512], FP32, tag="ps")
                    for c in range(4):
                        nc.tensor.matmul(
                            po[:],
                            xatT[b][:, c * L + lt * 128 : c * L + (lt + 1) * 128],
                            Wo_s[:, c * F : (c + 1) * F],
                            start=(c == 0),
                            stop=False,
                        )
                    nc.tensor.matmul(po[:], ones_s[:], bo_s[:], start=False, stop=True)
                    os_t = wpool.tile([128, 512], BF16, tag="os")
                    nc.scalar.copy(os_t[:], po[:])
                    nc.sync.dma_start(
                        out=out_d[b, lt * 128 : (lt + 1) * 128, :], in_=os_t[:]
                    )
                    yield

            bg = []

            def bg_step(n=1):
                for _ in range(n):
                    while bg:
                        try:
                            next(bg[0])
                            break
                        except StopIteration:
                            bg.pop(0)
                    else:
                        return

            # ---- window-pipelined phase B ----
            # window w: scores/exp/bias-mult for (h, b) = (w//2, w%2);
            # attn@v (+ per-qt softmax normalize) for the previous pair.
            for _ in qk_proj_steps(0, 0):
                pass
            bg.append(qk_proj_steps(0, 1))
            bg.append(v_proj_steps(0))
            bg.append(v_proj_steps(1))
            pending_proj = {1: (1, 0), 2: (1, 1), 5: (2, 0), 6: (2, 1),
                            9: (3, 0), 10: (3, 1)}

            def attnv_qt(h, b, exs, qt):
                U = psU.tile([128, 65], FP32, tag="u")
                for kt in range(8):
                    nc.tensor.matmul(
                        U[:],
                        exs[kt][:, qt * 128 : (qt + 1) * 128],
                        vA[b][:, kt * 520 + h * 65 : kt * 520 + (h + 1) * 65],
                        start=(kt == 0),
                        stop=(kt == 7),
                    )
                rc = wpool.tile([128, 1], FP32, tag="rc")
                nc.vector.reciprocal(rc[:], U[:, 64:65])
                nc.vector.tensor_scalar(
                    xan[b][:, qt * F + h * 64 : qt * F + (h + 1) * 64],
                    U[:, 0:64],
                    rc[:],
                    None,
                    op0=Mult,
                )

            prev = None  # (h, b, [ex tiles]) of the pair being drained
            for w in range(2 * H):
                h, b = w // 2, w % 2
                hp = (h % 2) * 64
                hc = (h // 2) * L
                if w in pending_proj:
                    bg.append(qk_proj_steps(*pending_proj[w]))
                if b == 0 and h >= 1 and h + 1 < H:
                    stage_bias(h + 1)
                exs = []
                for kt in range(8):
                    ps = psA.tile([128, 2 * 512], FP32, tag="ps")
                    for qc in range(2):
                        nc.tensor.matmul(
                            ps[:, qc * 512 : (qc + 1) * 512],
                            kT[b][hp : hp + 64, hc + kt * 128 : hc + (kt + 1) * 128],
                            qT[b][hp : hp + 64, hc + qc * 512 : hc + (qc + 1) * 512],
                            start=True,
                            stop=True,
                        )
                    es = espool.tile([128, 2 * 512], FP16, tag="es")
                    nc.scalar.activation(es[:], ps[:], Exp)
                    ex = epool.tile([128, 2 * 512], FP16, tag="ex")
                    nc.vector.tensor_tensor(
                        ex[:],
                        es[:],
                        bias_tiles[h][kt // 4][:, (kt % 4) * L : (kt % 4 + 1) * L],
                        Mult,
                    )
                    exs.append(ex)
                    if prev is not None and kt >= 1:
                        attnv_qt(prev[0], prev[1], prev[2], kt - 1)
                    bg_step(2 if w <= 1 else 1)
                if prev is not None:
                    attnv_qt(prev[0], prev[1], prev[2], 7)
                    if prev[0] % 2 == 1:
                        bg.append(transpose_steps(prev[1], (prev[0] - 1) // 2))
                    if prev[0] == H - 1:
                        bg.append(outproj_steps(prev[1]))
                prev = (h, b, exs)

            # drain: attn@v for the last pair (h=H-1, b=1)
            for qt in range(8):
                attnv_qt(prev[0], prev[1], prev[2], qt)
                bg_step(2)
            bg.append(transpose_steps(prev[1], 3))
            bg.append(outproj_steps(prev[1]))
            while bg:
                bg_step()

    nc.compile()
    return nc


_NC = None


def _get_nc():
    global _NC
    if _NC is None:
        _NC = _build()
    return _NC


def _prep_in_maps(inputs):
    bf16 = ml_dtypes.bfloat16
    xq = np.asarray(inputs["inputs_q"], dtype=np.float32)
    xkv = np.asarray(inputs["inputs_kv"], dtype=np.float32)
    Wq = np.asarray(inputs["Wq"], dtype=np.float32) * 0.125
    bq = np.asarray(inputs["bq"], dtype=np.float32) * 0.125
    Wk = np.asarray(inputs["Wk"], dtype=np.float32)
    bk = np.asarray(inputs["bk"], dtype=np.float32)
    Wv = np.asarray(inputs["Wv"], dtype=np.float32)
    Wo = np.asarray(inputs["Wo"], dtype=np.float32)
    bv_pad = np.zeros((128, F), dtype=np.float32)
    bv_pad[0] = np.asarray(inputs["bv"], dtype=np.float32)
    bo_pad = np.zeros((128, F), dtype=np.float32)
    bo_pad[0] = np.asarray(inputs["bo"], dtype=np.float32)
    onesrow = np.zeros((128, 128), dtype=np.float32)
    onesrow[0] = 1.0
    toe = np.asarray(inputs["toeplitz"], dtype=np.float32)

    def pack_x(x):
        # [B, L, F] -> xT [B, F, L] -> partition-major [B, 128, 4*L]
        xT = x.transpose(0, 2, 1).reshape(B, 4, 128, L)
        return np.ascontiguousarray(xT.transpose(0, 2, 1, 3)).reshape(
            B, 128, 4 * L
        ).astype(bf16)

    def pack_w(w):
        # [F, F] -> partition-major [128, 4*F]
        return np.ascontiguousarray(
            w.reshape(4, 128, F).transpose(1, 0, 2)
        ).reshape(128, 4 * F).astype(bf16)

    xqT = pack_x(xq)
    xkvT = pack_x(xkv)
    WqP, WkP, WvP, WoP = pack_w(Wq), pack_w(Wk), pack_w(Wv), pack_w(Wo)

    coords = np.arange(L)
    xi, yi = coords // NY, coords % NY
    dx = xi[:, None] - xi[None, :] + NX
    dy = yi[:, None] - yi[None, :] + NY
    idx = dx * (2 * NY) + dy  # [L(q), L(k)]
    bias = toe[:, idx]  # [H, L(q), L(k)]
    biasT = np.exp(bias.transpose(0, 2, 1))  # [H, L(k), L(q)]
    # partition-major [H, 2, 128, 4*L]
    biasT = np.ascontiguousarray(
        biasT.reshape(H, 2, 4, 128, L).transpose(0, 1, 3, 2, 4)
    ).reshape(H, 2, 128, 4 * L).astype(np.float16)

    in_maps = []
    for i in range(NCORES):
        sl = slice(i * BPC, (i + 1) * BPC)
        in_maps.append(
            {
                "xqT": np.ascontiguousarray(xqT[sl]),
                "xkvT": np.ascontiguousarray(xkvT[sl]),
                "Wq": WqP, "Wk": WkP, "Wv": WvP, "Wo": WoP,
                "bq": bq, "bk": bk,
                "bv": bv_pad.astype(bf16), "bo": bo_pad.astype(bf16),
                "biasT": biasT,
                "ones": onesrow.astype(bf16),
            }
        )
    return in_maps


def _run(inputs, trace=False):
    from concourse.bass_interp import get_hw_module

    nc = _get_nc()
    in_maps = _prep_in_maps(inputs)
    old_m = nc.m
    nc.m = get_hw_module(nc.m)
    try:
        res = run_bass_kernel_spmd(
            nc, in_maps, core_ids=list(range(NCORES)), trace=trace
        )
    finally:
        nc.m = old_m
    out = np.concatenate(
        [np.asarray(r["out"], dtype=np.float32) for r in res.results], axis=0
    )  # [B, L, F]
    return out.reshape(B, L, H, D), res


def kernel(**inputs) -> np.ndarray:
    out, _ = _run(inputs, trace=False)
    return out


# revision 40
# speedup vs baseline: 1.0484x; 1.0145x over previous
"""Trainium2 Bass kernel: multi-head attention with Toeplitz relative bias.

Problem: B=16, L=1024, F=512, H=8, D=64 ViT patch attention.
Sharding: data-parallel over batch, 2 batches per core across 8 cores.

Device-side design (per core, fully unrolled Tile program):
  - Host pre-packs every DRAM operand partition-major so each DMA descriptor
    covers a full 4-8KB partition row (small-descriptor overhead dominated
    the staging cost otherwise).
  - qT/kT computed transposed ([fout, L], W stationary); head pair stacked
    on partitions (64 rows each); scores use K=64 matmuls with matching
    partition bases, so kT needs no zero padding or memset.
  - v computed natural ([L, fout], xT stationary, bv via ones-row matmul).
  - Scores computed transposed [k, q] (k on partitions) so attn@v needs no
    transpose of the attention matrix. ACT does exp; DVE multiplies in the
    host-precomputed exp(bias) (fp16, 2x DVE mode).
  - attn@v in natural [q, d] layout with exp chunks as the stationary
    operand; softmax denominators accumulate into column 64 via a
    ones-column in vA, so normalization is a per-partition divide fused into
    one DVE tensor_scalar op.
  - The head loop is software-pipelined at (head, batch)-window granularity:
    window w computes scores/exp for pair (h,b) = (w//2, w%2) while the
    PREVIOUS pair's attn@v drains on the PE, so ACT (the ~1.1us/tile exp
    engine) and the PE both stay continuously busy.  QK projections are fed
    through a background queue one 4-matmul chain per slot.
  - x_attn is PE-transposed (identity trick) for the output projection; bo
    folded in via a ones-row matmul.
  - No max-subtraction in softmax: |scores| <~ 1.5 by construction.
"""

import sys

for _p in ("/opt/trn_rl_repo",):
    if _p not in sys.path:
        sys.path.insert(0, _p)

import numpy as np
import ml_dtypes

import concourse.bass as bass
import concourse.mybir as mybir
import concourse.tile as tile
from concourse import bacc
from concourse.bass_utils import run_bass_kernel_spmd
from concourse.masks import make_identity

B, L, F, H, D = 16, 1024, 512, 8, 64
NX, NY = 32, 32
NCORES = 8
BPC = B // NCORES  # batches per core
FP32 = mybir.dt.float32
BF16 = mybir.dt.bfloat16
FP16 = mybir.dt.float16
Exp = mybir.ActivationFunctionType.Exp
Identity = mybir.ActivationFunctionType.Identity
Mult = mybir.AluOpType.mult


def _build():
    nc = bacc.Bacc("TRN2", target_bir_lowering=False, debug=False)

    xqT_d = nc.dram_tensor("xqT", [BPC, 128, 4 * L], BF16, kind="ExternalInput").ap()
    xkvT_d = nc.dram_tensor("xkvT", [BPC, 128, 4 * L], BF16, kind="ExternalInput").ap()
    Wq_d = nc.dram_tensor("Wq", [128, 4 * F], BF16, kind="ExternalInput").ap()
    Wk_d = nc.dram_tensor("Wk", [128, 4 * F], BF16, kind="ExternalInput").ap()
    Wv_d = nc.dram_tensor("Wv", [128, 4 * F], BF16, kind="ExternalInput").ap()
    Wo_d = nc.dram_tensor("Wo", [128, 4 * F], BF16, kind="ExternalInput").ap()
    bq_d = nc.dram_tensor("bq", [F], FP32, kind="ExternalInput").ap()
    bk_d = nc.dram_tensor("bk", [F], FP32, kind="ExternalInput").ap()
    bv_d = nc.dram_tensor("bv", [128, F], BF16, kind="ExternalInput").ap()
    bo_d = nc.dram_tensor("bo", [128, F], BF16, kind="ExternalInput").ap()
    biasT_d = nc.dram_tensor("biasT", [H, 2, 128, 4 * L], FP16, kind="ExternalInput").ap()
    ones_d = nc.dram_tensor("ones", [128, 128], BF16, kind="ExternalInput").ap()
    out_d = nc.dram_tensor("out", [BPC, L, F], BF16, kind="ExternalOutput").ap()

    with tile.TileContext(nc) as tc:
        with (
            tc.tile_pool(name="const", bufs=1) as cpool,
            tc.tile_pool(name="xin", bufs=2) as xpool,
            tc.tile_pool(name="qkv", bufs=2) as qpool,
            tc.tile_pool(name="bias", bufs=3) as bpool,
            tc.tile_pool(name="work", bufs=2) as wpool,
            tc.tile_pool(name="exp", bufs=17) as epool,
            tc.tile_pool(name="es", bufs=5) as espool,
            tc.tile_pool(name="psA", bufs=3, space="PSUM") as psA,
            tc.tile_pool(name="psU", bufs=2, space="PSUM") as psU,
        ):
            # ---- constants ----
            Wv_s = cpool.tile([128, 4 * F], BF16, tag="Wv")
            Wq_s = cpool.tile([128, 4 * F], BF16, tag="Wq")
            Wk_s = cpool.tile([128, 4 * F], BF16, tag="Wk")
            Wo_s = cpool.tile([128, 4 * F], BF16, tag="Wo")

            bias_tiles = {}

            def stage_bias(h):
                tiles = []
                for hh in range(2):
                    bt = bpool.tile([128, 4 * L], FP16, tag="bias")
                    nc.sync.dma_start(out=bt[:], in_=biasT_d[h, hh])
                    tiles.append(bt)
                bias_tiles[h] = tiles

            bq_s = cpool.tile([128, 4], FP32, tag="bq")
            bk_s = cpool.tile([128, 4], FP32, tag="bk")

            qT, kT, vA, xan, xatT, xq, xkv = [], [], [], [], [], [], []
            for b in range(BPC):
                xq_t = xpool.tile([128, 4 * L], BF16, tag="xq")
                xkv_t = xpool.tile([128, 4 * L], BF16, tag="xkv")
                nc.sync.dma_start(out=xkv_t[:], in_=xkvT_d[b])
                if b == 0:
                    nc.sync.dma_start(out=Wq_s[:], in_=Wq_d)
                nc.sync.dma_start(out=xq_t[:], in_=xqT_d[b])
                if b == 0:
                    nc.sync.dma_start(out=Wk_s[:], in_=Wk_d)
                    for b_s, b_d in ((bq_s, bq_d), (bk_s, bk_d)):
                        nc.sync.dma_start(
                            out=b_s[:], in_=b_d.rearrange("(c p) -> p c", p=128)
                        )
                xq.append(xq_t)
                xkv.append(xkv_t)
                if b == 0:
                    stage_bias(0)
                    nc.sync.dma_start(out=Wv_s[:], in_=Wv_d)
                    ones_s = cpool.tile([128, 128], BF16, tag="ones")
                    nc.sync.dma_start(out=ones_s[:], in_=ones_d)
                    bv_s = cpool.tile([128, F], BF16, tag="bv")
                    nc.sync.dma_start(out=bv_s[:], in_=bv_d)
                qT_t = qpool.tile([128, 4 * L], BF16, tag="qT")
                kT_t = qpool.tile([128, 4 * L], BF16, tag="kT")
                vA_t = qpool.tile([128, 8 * 8 * 65], FP16, tag="vA")
                xan_t = qpool.tile([128, 8 * F], BF16, tag="xan")
                xatT_t = qpool.tile([128, 4 * L], BF16, tag="xatT")
                qT.append(qT_t)
                kT.append(kT_t)
                vA.append(vA_t)
                xan.append(xan_t)
                xatT.append(xatT_t)
            stage_bias(1)
            load_late = [(Wo_s, Wo_d)]
            bo_s = cpool.tile([128, F], BF16, tag="bo")
            ident = cpool.tile([128, 128], BF16, tag="ident")
            make_identity(nc, ident[:])
            nc.sync.dma_start(out=Wo_s[:], in_=Wo_d)
            nc.sync.dma_start(out=bo_s[:], in_=bo_d)

            # ---- background PE work generators ----
            def v_proj_steps(b):
                # ones column for the softmax denominator accumulation
                nc.gpsimd.memset(
                    vA[b][:].rearrange("p (t h w) -> p t h w", t=8, h=8)[:, :, :, 64:65],
                    1.0,
                )
                # v natural (+bv via ones-row matmul): xT stationary, Wv moving
                for lt in range(8):
                    pv = psA.tile([128, 512], FP32, tag="ps")
                    for kc in range(4):
                        nc.tensor.matmul(
                            pv[:],
                            xkv[b][:, kc * L + lt * 128 : kc * L + (lt + 1) * 128],
                            Wv_s[:, kc * F : (kc + 1) * F],
                            start=(kc == 0),
                            stop=False,
                        )
                    nc.tensor.matmul(pv[:], ones_s[:], bv_s[:], start=False, stop=True)
                    nc.scalar.activation(
                        vA[b][:, lt * 520 : (lt + 1) * 520].rearrange(
                            "p (h w) -> p h w", h=8
                        )[:, :, 0:64],
                        pv[:].rearrange("p (h w) -> p h w", h=8),
                        Identity,
                        bias=0.0,
                    )
                    yield

            def qk_proj_steps(fo, b):
                # qT/kT transposed [fout, L]; head pair stacked on partitions
                for which, w_s, b_s, x_t, dst in (
                    ("q", Wq_s, bq_s, xq[b], qT[b]),
                    ("k", Wk_s, bk_s, xkv[b], kT[b]),
                ):
                    pq = psA.tile([128, 2 * 512], FP32, tag="ps")
                    for lc in range(2):
                        for kc in range(4):
                            nc.tensor.matmul(
                                pq[:, lc * 512 : (lc + 1) * 512],
                                w_s[:, kc * F + fo * 128 : kc * F + (fo + 1) * 128],
                                x_t[:, kc * L + lc * 512 : kc * L + (lc + 1) * 512],
                                start=(kc == 0),
                                stop=(kc == 3),
                            )
                        if lc == 0:
                            yield
                    nc.vector.tensor_scalar_add(
                        dst[:, fo * L : (fo + 1) * L], pq[:], b_s[:, fo : fo + 1]
                    )
                    yield

            def transpose_steps(b, c):
                # transpose x_attn chunk c (heads 2c, 2c+1) via identity trick
                for qt in range(8):
                    pt = psA.tile([128, 512], BF16, tag="ps")
                    nc.tensor.transpose(
                        pt[:, 0:128],
                        xan[b][:, qt * F + c * 128 : qt * F + (c + 1) * 128],
                        ident[:],
                    )
                    nc.vector.tensor_copy(
                        xatT[b][:, c * L + qt * 128 : c * L + (qt + 1) * 128],
                        pt[:, 0:128],
                    )
                    if qt % 4 == 3:
                        yield

            def outproj_steps(b):
                for lt in range(8):
                    po = psA.tile([128, 512], FP32, tag="ps")
                    for c in range(4):
                        nc.tensor.matmul(
                            po[:],
                            xatT[b][:, c * L + lt * 128 : c * L + (lt + 1) * 128],
                            Wo_s[:, c * F : (c + 1) * F],
                            start=(c == 0),
                            stop=False,
                        )
                    nc.tensor.matmul(po[:], ones_s[:], bo_s[:], start=False, stop=True)
                    os_t = wpool.tile([128, 512], BF16, tag="os")
                    nc.scalar.copy(os_t[:], po[:])
                    nc.sync.dma_start(
                        out=out_d[b, lt * 128 : (lt + 1) * 128, :], in_=os_t[:]
                    )
                    yield

            bg = []

            def bg_step(n=1):
                for _ in range(n):
                    while bg:
                        try:
                            next(bg[0])
                            break
                        except StopIteration:
                            bg.pop(0)
                    else:
                        return

            # ---- window-pipelined phase B ----
            # window w: scores/exp/bias-mult for (h, b) = (w//2, w%2);
            # attn@v (+ per-qt softmax normalize) for the previous pair.
            for _ in qk_proj_steps(0, 0):
                pass
            bg.append(qk_proj_steps(0, 1))
            bg.append(v_proj_steps(0))
            bg.append(v_proj_steps(1))
            pending_proj = {1: (1, 0), 2: (1, 1), 5: (2, 0), 6: (2, 1),
                            9: (3, 0), 10: (3, 1)}

            quad_state = {}

            def attnv_qt(h, b, exs, qt):
                if qt % 4 == 0:
                    U_t = psU.tile([128, 4 * 65], FP32, tag="u")
                    quad_state[0] = U_t
                U = quad_state[0]
                qo = (qt % 4) * 65
                for kt in range(8):
                    nc.tensor.matmul(
                        U[:, qo : qo + 65],
                        exs[kt][:, qt * 128 : (qt + 1) * 128],
                        vA[b][:, kt * 520 + h * 65 : kt * 520 + (h + 1) * 65],
                        start=(kt == 0),
                        stop=(kt == 7),
                    )
                if qt % 4 == 3:
                    rc = wpool.tile([128, 4], FP32, tag="rc")
                    nc.vector.reciprocal(
                        rc[:], U[:].rearrange("p (q w) -> p q w", q=4)[:, :, 64:65]
                    )
                    for j in range(4):
                        q0 = qt - 3 + j
                        nc.vector.tensor_scalar(
                            xan[b][:, q0 * F + h * 64 : q0 * F + (h + 1) * 64],
                            U[:, j * 65 : j * 65 + 64],
                            rc[:, j : j + 1],
                            None,
                            op0=Mult,
                        )

            prev = None  # (h, b, [ex tiles]) of the pair being drained
            for w in range(2 * H):
                h, b = w // 2, w % 2
                hp = (h % 2) * 64
                hc = (h // 2) * L
                if w in pending_proj:
                    bg.append(qk_proj_steps(*pending_proj[w]))
                if b == 0 and h >= 1 and h + 1 < H:
                    stage_bias(h + 1)
                exs = []
                for kt in range(8):
                    ps = psA.tile([128, 2 * 512], FP32, tag="ps")
                    for qc in range(2):
                        nc.tensor.matmul(
                            ps[:, qc * 512 : (qc + 1) * 512],
                            kT[b][hp : hp + 64, hc + kt * 128 : hc + (kt + 1) * 128],
                            qT[b][hp : hp + 64, hc + qc * 512 : hc + (qc + 1) * 512],
                            start=True,
                            stop=True,
                        )
                    es = espool.tile([128, 2 * 512], FP16, tag="es")
                    nc.scalar.activation(es[:], ps[:], Exp)
                    ex = epool.tile([128, 2 * 512], FP16, tag="ex")
                    eng = nc.gpsimd if kt in (2, 5) else nc.vector
                    eng.tensor_tensor(
                        ex[:],
                        es[:],
                        bias_tiles[h][kt // 4][:, (kt % 4) * L : (kt % 4 + 1) * L],
                        Mult,
                    )
                    exs.append(ex)
                    if prev is not None and kt >= 1:
                        attnv_qt(prev[0], prev[1], prev[2], kt - 1)
                    bg_step(2 if w <= 1 else 1)
                if prev is not None:
                    attnv_qt(prev[0], prev[1], prev[2], 7)
                    if prev[0] % 2 == 1:
                        bg.append(transpose_steps(prev[1], (prev[0] - 1) // 2))
                    if prev[0] == H - 1:
                        bg.append(outproj_steps(prev[1]))
                prev = (h, b, exs)

            # drain: attn@v for the last pair (h=H-1, b=1)
            for qt in range(8):
                attnv_qt(prev[0], prev[1], prev[2], qt)
                bg_step(2)
            bg.append(transpose_steps(prev[1], 3))
            bg.append(outproj_steps(prev[1]))
            while bg:
                bg_step()

    nc.compile()
    return nc


_NC = None


def _get_nc():
    global _NC
    if _NC is None:
        _NC = _build()
    return _NC


def _prep_in_maps(inputs):
    bf16 = ml_dtypes.bfloat16
    xq = np.asarray(inputs["inputs_q"], dtype=np.float32)
    xkv = np.asarray(inputs["inputs_kv"], dtype=np.float32)
    Wq = np.asarray(inputs["Wq"], dtype=np.float32) * 0.125
    bq = np.asarray(inputs["bq"], dtype=np.float32) * 0.125
    Wk = np.asarray(inputs["Wk"], dtype=np.float32)
    bk = np.asarray(inputs["bk"], dtype=np.float32)
    Wv = np.asarray(inputs["Wv"], dtype=np.float32)
    Wo = np.asarray(inputs["Wo"], dtype=np.float32)
    bv_pad = np.zeros((128, F), dtype=np.float32)
    bv_pad[0] = np.asarray(inputs["bv"], dtype=np.float32)
    bo_pad = np.zeros((128, F), dtype=np.float32)
    bo_pad[0] = np.asarray(inputs["bo"], dtype=np.float32)
    onesrow = np.zeros((128, 128), dtype=np.float32)
    onesrow[0] = 1.0
    toe = np.asarray(inputs["toeplitz"], dtype=np.float32)

    def pack_x(x):
        # [B, L, F] -> xT [B, F, L] -> partition-major [B, 128, 4*L]
        xT = x.transpose(0, 2, 1).reshape(B, 4, 128, L)
        return np.ascontiguousarray(xT.transpose(0, 2, 1, 3)).reshape(
            B, 128, 4 * L
        ).astype(bf16)

    def pack_w(w):
        # [F, F] -> partition-major [128, 4*F]
        return np.ascontiguousarray(
            w.reshape(4, 128, F).transpose(1, 0, 2)
        ).reshape(128, 4 * F).astype(bf16)

    xqT = pack_x(xq)
    xkvT = pack_x(xkv)
    WqP, WkP, WvP, WoP = pack_w(Wq), pack_w(Wk), pack_w(Wv), pack_w(Wo)

    coords = np.arange(L)
    xi, yi = coords // NY, coords % NY
    dx = xi[:, None] - xi[None, :] + NX
    dy = yi[:, None] - yi[None, :] + NY
    idx = dx * (2 * NY) + dy  # [L(q), L(k)]
    bias = toe[:, idx]  # [H, L(q), L(k)]
    biasT = np.exp(bias.transpose(0, 2, 1))  # [H, L(k), L(q)]
    # partition-major [H, 2, 128, 4*L]
    biasT = np.ascontiguousarray(
        biasT.reshape(H, 2, 4, 128, L).transpose(0, 1, 3, 2, 4)
    ).reshape(H, 2, 128, 4 * L).astype(np.float16)

    in_maps = []
    for i in range(NCORES):
        sl = slice(i * BPC, (i + 1) * BPC)
        in_maps.append(
            {
                "xqT": np.ascontiguousarray(xqT[sl]),
                "xkvT": np.ascontiguousarray(xkvT[sl]),
                "Wq": WqP, "Wk": WkP, "Wv": WvP, "Wo": WoP,
                "bq": bq, "bk": bk,
                "bv": bv_pad.astype(bf16), "bo": bo_pad.astype(bf16),
                "biasT": biasT,
                "ones": onesrow.astype(bf16),
            }
        )
    return in_maps


def _run(inputs, trace=False):
    from concourse.bass_interp import get_hw_module

    nc = _get_nc()
    in_maps = _prep_in_maps(inputs)
    old_m = nc.m
    nc.m = get_hw_module(nc.m)
    try:
        res = run_bass_kernel_spmd(
            nc, in_maps, core_ids=list(range(NCORES)), trace=trace
        )
    finally:
        nc.m = old_m
    out = np.concatenate(
        [np.asarray(r["out"], dtype=np.float32) for r in res.results], axis=0
    )  # [B, L, F]
    return out.reshape(B, L, H, D), res


def kernel(**inputs) -> np.ndarray:
    out, _ = _run(inputs, trace=False)
    return out


# revision 41
# speedup vs baseline: 1.1201x; 1.0683x over previous
"""Trainium2 Bass kernel: multi-head attention with Toeplitz relative bias.

Problem: B=16, L=1024, F=512, H=8, D=64 ViT patch attention.
Sharding: data-parallel over batch, 2 batches per core across 8 cores.

Device-side design (per core, fully unrolled Tile program, per-batch pipeline):
  - Host pre-transposes inputs to xT [F, L] (bf16) so the F-contraction of
    every projection has F on SBUF partitions with contiguous DMA loads.
  - qT/kT computed transposed ([fout, L], W stationary); v computed natural
    ([L, fout], xT stationary, bv folded in via a ones-row matmul).
  - Scores computed transposed [k, q] (k on partitions) so attn@v needs no
    transpose of the attention matrix. Host-gathered Toeplitz bias (bf16) is
    added by DVE straight off PSUM; ACT does exp.
  - attn@v computed in natural [q, d] layout with exp chunks as the
    stationary operand; softmax denominators accumulate into column 64 via a
    ones-column matmul, so normalization is a per-partition divide fused into
    one DVE tensor_scalar op.
  - x_attn is PE-transposed (identity trick) for the output projection; bo is
    folded in via a ones-row matmul.
  - No max-subtraction in softmax: |scores| <~ 1.5 by construction
    (0.02-scale weights), exp is far from overflow.
"""

import os
import sys

import numpy as np

for _p in ("/opt/trn_rl_repo",):
    if _p not in sys.path:
        sys.path.insert(0, _p)

import ml_dtypes

import concourse.bass as bass
import concourse.mybir as mybir
import concourse.tile as tile
from concourse import bacc
from concourse.bass_utils import run_bass_kernel_spmd
from concourse.masks import make_identity

B, L, F, H, D = 16, 1024, 512, 8, 64
NX, NY = 32, 32
NCORES = 8
BPC = B // NCORES  # batches per core
FP32 = mybir.dt.float32
F32R = mybir.dt.float32r
BF16 = mybir.dt.bfloat16
FP16 = mybir.dt.float16
Exp = mybir.ActivationFunctionType.Exp
Identity = mybir.ActivationFunctionType.Identity
Add = mybir.AluOpType.add
Mult = mybir.AluOpType.mult


def _build():
    nc = bacc.Bacc("TRN2", target_bir_lowering=False, debug=False)

    xqT_d = nc.dram_tensor("xqT", [BPC, 128, 4 * L], BF16, kind="ExternalInput").ap()
    xkvT_d = nc.dram_tensor("xkvT", [BPC, 128, 4 * L], BF16, kind="ExternalInput").ap()
    Wq_d = nc.dram_tensor("Wq", [128, 4 * F], BF16, kind="ExternalInput").ap()
    Wk_d = nc.dram_tensor("Wk", [128, 4 * F], BF16, kind="ExternalInput").ap()
    Wv_d = nc.dram_tensor("Wv", [128, 4 * F], BF16, kind="ExternalInput").ap()
    Wo_d = nc.dram_tensor("Wo", [128, 4 * F], BF16, kind="ExternalInput").ap()
    bq_d = nc.dram_tensor("bq", [F], FP32, kind="ExternalInput").ap()
    bk_d = nc.dram_tensor("bk", [F], FP32, kind="ExternalInput").ap()
    bv_d = nc.dram_tensor("bv", [128, F], BF16, kind="ExternalInput").ap()
    bo_d = nc.dram_tensor("bo", [128, F], BF16, kind="ExternalInput").ap()
    biasT_d = nc.dram_tensor("biasT", [H, 2, 128, 4 * L], FP16, kind="ExternalInput").ap()
    ones_d = nc.dram_tensor("ones", [128, 128], BF16, kind="ExternalInput").ap()
    out_d = nc.dram_tensor("out", [BPC, L, F], BF16, kind="ExternalOutput").ap()

    with tile.TileContext(nc) as tc:
        with (
            tc.tile_pool(name="const", bufs=1) as cpool,
            tc.tile_pool(name="xin", bufs=2) as xpool,
            tc.tile_pool(name="qkv", bufs=2) as qpool,
            tc.tile_pool(name="bias", bufs=2) as bpool,
            tc.tile_pool(name="work", bufs=2) as wpool,
            tc.tile_pool(name="exp", bufs=17) as epool,
            tc.tile_pool(name="es", bufs=3) as espool,
            tc.tile_pool(name="psA", bufs=3, space="PSUM") as psA,
            tc.tile_pool(name="psU", bufs=2, space="PSUM") as psU,
        ):
            # ---- constants: weights, biases, ones, identity ----
            # v-projection dependencies stream first so the PE starts ASAP
            Wv_s = cpool.tile([128, 4 * F], BF16, tag="Wv")
            Wq_s = cpool.tile([128, 4 * F], BF16, tag="Wq")
            Wk_s = cpool.tile([128, 4 * F], BF16, tag="Wk")
            Wo_s = cpool.tile([128, 4 * F], BF16, tag="Wo")
            def load_w(w_s, w_d):
                nc.sync.dma_start(out=w_s[:], in_=w_d)
            nc.sync.dma_start(out=Wv_s[:], in_=Wv_d)
            ones_s = cpool.tile([128, 128], BF16, tag="ones")
            nc.sync.dma_start(out=ones_s[:], in_=ones_d)
            bv_s = cpool.tile([128, F], BF16, tag="bv")
            nc.sync.dma_start(out=bv_s[:], in_=bv_d)

            qT, kT, vA, xan, xatT, xq, xkv = [], [], [], [], [], [], []
            for b in range(BPC):
                # ---- phase A: load inputs + projections ----
                xq_t = xpool.tile([128, 4 * L], BF16, tag="xq")
                xkv_t = xpool.tile([128, 4 * L], BF16, tag="xkv")
                nc.sync.dma_start(out=xkv_t[:], in_=xkvT_d[b])
                if b == 0:
                    load_w(Wq_s, Wq_d)
                    load_w(Wk_s, Wk_d)
                    load_w(Wo_s, Wo_d)
                    bq_s = cpool.tile([128, 4], FP32, tag="bq")
                    bk_s = cpool.tile([128, 4], FP32, tag="bk")
                    for b_s, b_d in ((bq_s, bq_d), (bk_s, bk_d)):
                        nc.sync.dma_start(
                            out=b_s[:], in_=b_d.rearrange("(c p) -> p c", p=128)
                        )
                    bo_s = cpool.tile([128, F], BF16, tag="bo")
                    nc.sync.dma_start(out=bo_s[:], in_=bo_d)
                    ident = cpool.tile([128, 128], BF16, tag="ident")
                    make_identity(nc, ident[:])
                qT_t = qpool.tile([128, 4 * L], BF16, tag="qT")
                kT_t = qpool.tile([128, 8 * L], BF16, tag="kT")
                nc.gpsimd.memset(kT_t[:], 0.0)
                vA_t = qpool.tile([128, 8 * 8 * 65], FP16, tag="vA")
                qT.append(qT_t)
                kT.append(kT_t)
                vA.append(vA_t)
                xq.append(xq_t)
                xkv.append(xkv_t)
                xan_t = qpool.tile([128, 8 * F], BF16, tag="xan")
                xatT_t = qpool.tile([128, 4 * L], BF16, tag="xatT")
                xan.append(xan_t)
                xatT.append(xatT_t)

                # v natural (+bv via ones-row matmul): xT stationary, Wv moving
                for lt in range(8):
                    pv = psA.tile([128, 512], FP32, tag="ps")
                    for kc in range(4):
                        nc.tensor.matmul(
                            pv[:],
                            xkv_t[:, kc * L + lt * 128 : kc * L + (lt + 1) * 128],
                            Wv_s[:, kc * F : (kc + 1) * F],
                            start=(kc == 0),
                            stop=False,
                        )
                    nc.tensor.matmul(
                        pv[:], ones_s[:], bv_s[:], start=False, stop=True
                    )
                    nc.scalar.activation(
                        vA_t[:, lt * 520 : (lt + 1) * 520].rearrange(
                            "p (h w) -> p h w", h=8
                        )[:, :, 0:64],
                        pv[:].rearrange("p (h w) -> p h w", h=8),
                        Identity,
                        bias=0.0,
                    )

                nc.gpsimd.memset(
                    vA_t[:].rearrange("p (t h w) -> p t h w", t=8, h=8)[:, :, :, 64:65],
                    1.0,
                )
                nc.sync.dma_start(out=xq_t[:], in_=xqT_d[b])


            def qk_proj(fo):
                for b in range(BPC):
                    for which, w_s, b_s, x_t in (
                        ("q", Wq_s, bq_s, xq[b]),
                        ("k", Wk_s, bk_s, xkv[b]),
                    ):
                        for lc in range(2):
                            pq = psA.tile([128, 512], FP32, tag="ps")
                            for kc in range(4):
                                nc.tensor.matmul(
                                    pq[:],
                                    w_s[:, kc * F + fo * 128 : kc * F + (fo + 1) * 128],
                                    x_t[:, kc * L + lc * 512 : kc * L + (lc + 1) * 512],
                                    start=(kc == 0),
                                    stop=(kc == 3),
                                )
                            if which == "q":
                                nc.vector.tensor_scalar_add(
                                    qT[b][:, fo * L + lc * 512 : fo * L + (lc + 1) * 512],
                                    pq[:],
                                    b_s[:, fo : fo + 1],
                                )
                            else:
                                # split the head pair into zero-padded blocks so
                                # the scores matmul gets a full K=128
                                for hh in range(2):
                                    hdst = 2 * fo + hh
                                    nc.vector.tensor_scalar_add(
                                        kT[b][
                                            hh * 64 : (hh + 1) * 64,
                                            hdst * L + lc * 512 : hdst * L + (lc + 1) * 512,
                                        ],
                                        pq[hh * 64 : (hh + 1) * 64, :],
                                        b_s[hh * 64 : (hh + 1) * 64, fo : fo + 1],
                                    )
            def emit_C(b):
                # ---- transpose x_attn for the output projection ----
                for c in range(4):
                    for qt in range(8):
                        pt = psA.tile([128, 512], BF16, tag="ps")
                        nc.tensor.transpose(
                            pt[:, 0:128],
                            xan[b][:, qt * F + c * 128 : qt * F + (c + 1) * 128],
                            ident[:],
                        )
                        nc.vector.tensor_copy(
                            xatT[b][:, c * L + qt * 128 : c * L + (qt + 1) * 128],
                            pt[:, 0:128],
                        )

                # ---- phase C: output projection (+bo via ones-row matmul) ----
                for lt in range(8):
                    po = psA.tile([128, 512], FP32, tag="ps")
                    for c in range(4):
                        nc.tensor.matmul(
                            po[:],
                            xatT[b][:, c * L + lt * 128 : c * L + (lt + 1) * 128],
                            Wo_s[:, c * F : (c + 1) * F],
                            start=(c == 0),
                            stop=False,
                        )
                    nc.tensor.matmul(
                        po[:], ones_s[:], bo_s[:], start=False, stop=True
                    )
                    os_t = wpool.tile([128, 512], BF16, tag="os")
                    nc.scalar.copy(os_t[:], po[:])
                    nc.sync.dma_start(out=out_d[b, lt * 128 : (lt + 1) * 128, :], in_=os_t[:])

            # ---- phase B: attention, batches interleaved per head so the PE
            # never waits on the exp pipeline and bias staging is shared.
            # qT/kT projections for fout chunk h//2 are emitted just before the
            # heads that consume them, filling PE while ACT drains exp work ----
            for h in range(H):
                if h % 2 == 0:
                    qk_proj(h // 2)
                hp = (h % 2) * 64  # partition offset within fout chunk
                hc = (h // 2) * L  # column offset of fout chunk
                bias_tiles = []
                for hh in range(2):  # exp(bias) half-head staging
                    bias_t = bpool.tile([128, 4 * L], FP16, tag="bias")
                    nc.sync.dma_start(out=bias_t[:], in_=biasT_d[h, hh])
                    bias_tiles.append(bias_t)
                ex_all = {}
                for b in range(BPC):
                    ex_tiles = []
                    for kt in range(8):
                        ps = psA.tile([128, 2 * 512], FP32, tag="ps")
                        for qc in range(2):
                            nc.tensor.matmul(
                                ps[:, qc * 512 : (qc + 1) * 512],
                                kT[b][:, h * L + kt * 128 : h * L + (kt + 1) * 128],
                                qT[b][:, hc + qc * 512 : hc + (qc + 1) * 512],
                                start=True,
                                stop=True,
                            )
                        es = espool.tile([128, 2 * 512], FP16, tag="es")
                        nc.scalar.activation(es[:], ps[:], Exp)
                        ex = epool.tile([128, 2 * 512], FP16, tag="ex")
                        nc.vector.tensor_tensor(
                            ex[:],
                            es[:],
                            bias_tiles[kt // 4][:, (kt % 4) * L : (kt % 4 + 1) * L],
                            Mult,
                        )
                        ex_tiles.append(ex)
                    ex_all[b] = ex_tiles
                for b in range(BPC):
                    # attn @ v_aug in natural [q, d] layout; denom in col 64
                    for qt in range(8):
                        U = psU.tile([128, 65], FP32, tag="u")
                        for kt in range(8):
                            nc.tensor.matmul(
                                U[:],
                                ex_all[b][kt][:, qt * 128 : (qt + 1) * 128],
                                vA[b][:, kt * 520 + h * 65 : kt * 520 + (h + 1) * 65],
                                start=(kt == 0),
                                stop=(kt == 7),
                            )
                        rc = wpool.tile([128, 1], FP32, tag="rc")
                        nc.vector.reciprocal(rc[:], U[:, 64:65])
                        nc.vector.tensor_scalar(
                            xan[b][:, qt * F + h * 64 : qt * F + (h + 1) * 64],
                            U[:, 0:64],
                            rc[:],
                            None,
                            op0=Mult,
                        )
                    if h == H - 1:
                        emit_C(b)

    nc.compile()
    return nc


_NC = None


def _get_nc():
    global _NC
    if _NC is None:
        _NC = _build()
    return _NC


def _prep_in_maps(inputs):
    bf16 = ml_dtypes.bfloat16
    xq = np.asarray(inputs["inputs_q"], dtype=np.float32)
    xkv = np.asarray(inputs["inputs_kv"], dtype=np.float32)
    Wq = (np.asarray(inputs["Wq"], dtype=np.float32) * 0.125).astype(bf16)
    bq = np.asarray(inputs["bq"], dtype=np.float32) * 0.125
    Wk = np.asarray(inputs["Wk"], dtype=np.float32).astype(bf16)
    bk = np.asarray(inputs["bk"], dtype=np.float32)
    Wv = np.asarray(inputs["Wv"], dtype=np.float32).astype(bf16)
    bv_pad = np.zeros((128, F), dtype=np.float32)
    bv_pad[0] = np.asarray(inputs["bv"], dtype=np.float32)
    Wo = np.asarray(inputs["Wo"], dtype=np.float32).astype(bf16)
    bo_pad = np.zeros((128, F), dtype=np.float32)
    bo_pad[0] = np.asarray(inputs["bo"], dtype=np.float32)
    onesrow = np.zeros((128, 128), dtype=np.float32)
    onesrow[0] = 1.0
    toe = np.asarray(inputs["toeplitz"], dtype=np.float32)

    def pack_x(x):
        xT = x.transpose(0, 2, 1).reshape(B, 4, 128, L)
        return np.ascontiguousarray(xT.transpose(0, 2, 1, 3)).reshape(
            B, 128, 4 * L
        ).astype(bf16)

    def pack_w(w):
        return np.ascontiguousarray(
            np.asarray(w, dtype=np.float32).reshape(4, 128, F).transpose(1, 0, 2)
        ).reshape(128, 4 * F).astype(bf16)

    xqT = pack_x(xq)
    xkvT = pack_x(xkv)
    Wq, Wk, Wv, Wo = pack_w(Wq), pack_w(Wk), pack_w(Wv), pack_w(Wo)

    coords = np.arange(L)
    xi, yi = coords // NY, coords % NY
    dx = xi[:, None] - xi[None, :] + NX
    dy = yi[:, None] - yi[None, :] + NY
    idx = dx * (2 * NY) + dy  # [L(q), L(k)]
    bias = toe[:, idx]  # [H, L(q), L(k)]
    biasT = np.exp(bias.transpose(0, 2, 1))  # [H, L(k), L(q)]
    biasT = np.ascontiguousarray(
        biasT.reshape(H, 2, 4, 128, L).transpose(0, 1, 3, 2, 4)
    ).reshape(H, 2, 128, 4 * L).astype(np.float16)

    in_maps = []
    for i in range(NCORES):
        sl = slice(i * BPC, (i + 1) * BPC)
        in_maps.append(
            {
                "xqT": np.ascontiguousarray(xqT[sl]),
                "xkvT": np.ascontiguousarray(xkvT[sl]),
                "Wq": Wq, "Wk": Wk, "Wv": Wv, "Wo": Wo,
                "bq": bq, "bk": bk, "bv": bv_pad.astype(bf16), "bo": bo_pad.astype(bf16),
                "biasT": biasT,
                "ones": onesrow.astype(bf16),
            }
        )
    return in_maps


def _run(inputs, trace=False):
    from concourse.bass_interp import get_hw_module

    nc = _get_nc()
    in_maps = _prep_in_maps(inputs)
    old_m = nc.m
    nc.m = get_hw_module(nc.m)
    try:
        res = run_bass_kernel_spmd(
            nc, in_maps, core_ids=list(range(NCORES)), trace=trace
        )
    finally:
        nc.m = old_m
    out = np.concatenate(
        [np.asarray(r["out"], dtype=np.float32) for r in res.results], axis=0
    )  # [B, L, F]
    return out.reshape(B, L, H, D), res


def kernel(**inputs) -> np.ndarray:
    out, _ = _run(inputs, trace=False)
    return out

